# revision 12
# baseline (speedup 1.0000x reference)
"""Linear-attention Trainium2 kernel (8 NeuronCores, SPMD).

Sharding: batch (4) x head-group (2). Core i handles batch i//2, heads
[8*(i%2), 8*(i%2)+8). Each core computes its partial output through Wo;
the host sums the two partials per batch and adds bo.

Per-core dataflow (all matmuls in float32r):
  xT = x[b].T                                   [1024, 4096]   (host transpose)
  Q^T = Wq_g^T-contract xT  (PE, f on parts)    [512, 4096]    d on partitions
  expQ^T = exp(Q^T + bq)    (ACT, bias/part)
  sQ    = blockdiag-ones matmul colsums         [8, 4096]
  K     = xT^T-contract Wk_g (natural layout)   [4096, 512]    s on partitions
  expK  = exp(K + bk)       (ACT; bias via k=1 outer-product matmul)
  V'    = (V + bv) * 1/rowsum(expK) per head    (DVE tensor_scalar per head)
  KV_h  = expK_h^T @ V'_h   (PE, accumulated in PSUM over all of S)
  out^T_h = KV_h^T-contract expQ^T_h, then * (1/sQ) via DMA-broadcast + DVE
  y_partial = out^T^T-contract Wo_g             [4096, 1024]
"""

import numpy as np

B, S, DM, H = 4, 4096, 1024, 16
HD = 64
GROUPS = 2
DLOC = DM // GROUPS   # 512 channels per core
HLOC = H // GROUPS    # 8 heads per core
NCORES = B * GROUPS   # 8
SC = 512              # sequence chunk


def make_consts():
    ones1 = np.ones((1, 128), np.float32)
    ones8 = np.zeros((128, 4 * HLOC), np.float32)
    for dt_ in range(4):  # pair-tile index
        for sub in range(2):
            ones8[64 * sub:64 * (sub + 1), dt_ * HLOC + 2 * dt_ + sub] = 1.0
    return ones1, ones8


def kv_region(h):
    """(half, row_base, col_base) of KV_h inside kv psum tile [128, 2, 512]."""
    return h // 4, 64 * (h % 2), 256 * ((h // 2) % 2) + 64 * (h % 4)


def build_bass(S_=S, n_devices=NCORES, repeat=1, dbg=False):
    from contextlib import ExitStack
    import concourse.bass as bass
    import concourse.bacc as bacc
    import concourse.mybir as mybir
    import concourse.tile as tile

    f32 = mybir.dt.float32
    f32r = mybir.dt.float32r
    Exp = mybir.ActivationFunctionType.Exp
    X = mybir.AxisListType.X

    NCH = S_ // SC        # sequence chunks
    NPAIR = DLOC // 128   # 4 pair-tiles (2 heads each)
    NST = S_ // 128       # sequence tiles

    nc = bacc.Bacc("TRN2", target_bir_lowering=False, debug=False,
                   num_devices=n_devices)
    xT = nc.dram_tensor("xT", [DM, S_], f32r, kind="ExternalInput").ap()
    wq = nc.dram_tensor("wq", [DM, DLOC], f32r, kind="ExternalInput").ap()
    wk = nc.dram_tensor("wk", [DM, DLOC], f32r, kind="ExternalInput").ap()
    wv = nc.dram_tensor("wv", [DM, DLOC], f32r, kind="ExternalInput").ap()
    wo = nc.dram_tensor("wo", [DLOC, DM], f32r, kind="ExternalInput").ap()
    bq = nc.dram_tensor("bq", [DLOC], f32, kind="ExternalInput").ap()
    bk = nc.dram_tensor("bk", [1, DLOC], f32r, kind="ExternalInput").ap()
    bv = nc.dram_tensor("bv", [1, DLOC], f32r, kind="ExternalInput").ap()
    ones1 = nc.dram_tensor("ones1", [1, 128], f32r, kind="ExternalInput").ap()
    ones8 = nc.dram_tensor("ones8", [128, 4 * HLOC], f32r,
                           kind="ExternalInput").ap()
    y = nc.dram_tensor("y", [S_, DM], f32, kind="ExternalOutput").ap()
    NPAIR_ = DLOC // 128
    if dbg:
        d_expqt = nc.dram_tensor("d_expqt", [128, NPAIR_, S_], f32,
                                 kind="ExternalOutput").ap()
        d_recq = nc.dram_tensor("d_recq", [HLOC, S_], f32,
                                kind="ExternalOutput").ap()
        d_kv = nc.dram_tensor("d_kv", [128, 2, 512], f32,
                              kind="ExternalOutput").ap()
        d_ot = nc.dram_tensor("d_ot", [S_ // SC, 128, NPAIR_, SC], f32,
                              kind="ExternalOutput").ap()

    xTr = xT.rearrange("(tf p) s -> p tf s", p=128)

    def body(tc):
        ctx = ExitStack()
        with ctx:
            cons = ctx.enter_context(tc.tile_pool(name="cons", bufs=1))
            persist = ctx.enter_context(tc.tile_pool(name="persist", bufs=1))
            kvpsp = ctx.enter_context(
                tc.tile_pool(name="kvps", bufs=1, space="PSUM"))

            bqT = cons.tile([128, NPAIR], f32)
            nc.sync.dma_start(out=bqT, in_=bq.rearrange("(t p) -> p t", p=128))
            bk_sb = cons.tile([1, DLOC], f32r)
            nc.sync.dma_start(out=bk_sb, in_=bk)
            bv_sb = cons.tile([1, DLOC], f32r)
            nc.sync.dma_start(out=bv_sb, in_=bv)
            o1 = cons.tile([1, 128], f32r)
            nc.sync.dma_start(out=o1, in_=ones1)
            o8 = cons.tile([128, 4 * HLOC], f32r)
            nc.sync.dma_start(out=o8, in_=ones8)

            expQT = persist.tile([128, NPAIR, S_], f32r)
            recq = persist.tile([HLOC, S_], f32r)
            kvsb = persist.tile([128, 2, 512], f32r)
            kvA = kvpsp.tile([128, 512], f32, tag="kvA")
            kvB = kvpsp.tile([128, 512], f32, tag="kvB")

            # ---------------- phase 1 ----------------
            with ExitStack() as p1:
                wpool = p1.enter_context(tc.tile_pool(name="wqkv", bufs=1))
                xpool = p1.enter_context(tc.tile_pool(name="xc", bufs=2))
                ekpool = p1.enter_context(tc.tile_pool(name="ek", bufs=3))
                vnpool = p1.enter_context(tc.tile_pool(name="vn", bufs=2))
                smpool = p1.enter_context(tc.tile_pool(name="sm", bufs=4))
                qpsp = p1.enter_context(
                    tc.tile_pool(name="qps", bufs=2, space="PSUM"))
                sqpsp = p1.enter_context(
                    tc.tile_pool(name="sqps", bufs=1, space="PSUM"))
                pkvp = p1.enter_context(
                    tc.tile_pool(name="pkv", bufs=2, space="PSUM"))

                wq_sb = wpool.tile([128, 8, DLOC], f32r, tag="wq")
                nc.sync.dma_start(
                    out=wq_sb, in_=wq.rearrange("(tf p) d -> p tf d", p=128))
                wk_sb = wpool.tile([128, 8, DLOC], f32r, tag="wk")
                nc.sync.dma_start(
                    out=wk_sb, in_=wk.rearrange("(tf p) d -> p tf d", p=128))
                wv_sb = wpool.tile([128, 8, DLOC], f32r, tag="wv")
                nc.sync.dma_start(
                    out=wv_sb, in_=wv.rearrange("(tf p) d -> p tf d", p=128))

                for c in range(NCH):
                    xc = xpool.tile([128, 8, SC], f32r, tag="xc")
                    nc.sync.dma_start(out=xc,
                                      in_=xTr[:, :, c * SC:(c + 1) * SC])
                    # Q^T pair-tiles + exp + column sums
                    sqps = sqpsp.tile([HLOC, SC], f32, tag="sq")
                    for dt_ in range(NPAIR):
                        qps = qpsp.tile([128, SC], f32, tag="q")
                        for tf in range(8):
                            nc.tensor.matmul(
                                qps, wq_sb[:, tf, dt_ * 128:(dt_ + 1) * 128],
                                xc[:, tf, :],
                                start=(tf == 0), stop=(tf == 7))
                        eq = expQT[:, dt_, c * SC:(c + 1) * SC]
                        nc.scalar.activation(eq, qps, Exp,
                                             bias=bqT[:, dt_:dt_ + 1],
                                             scale=1.0)
                        nc.tensor.matmul(
                            sqps, o8[:, dt_ * HLOC:(dt_ + 1) * HLOC], eq,
                            start=(dt_ == 0), stop=(dt_ == NPAIR - 1))
                    with nc.allow_low_precision(reason="f32r rounding ok"):
                        nc.vector.reciprocal(
                            recq[:, c * SC:(c + 1) * SC], sqps)

                    # K / V / KV per 128-row sequence tile
                    for t in range(4):
                        st = c * 4 + t
                        kps = pkvp.tile([128, DLOC], f32, tag="pkv")
                        for tf in range(8):
                            nc.tensor.matmul(
                                kps, xc[:, tf, t * 128:(t + 1) * 128],
                                wk_sb[:, tf, :],
                                start=(tf == 0), stop=False)
                        nc.tensor.matmul(kps, o1, bk_sb,
                                         start=False, stop=True)
                        ek = ekpool.tile([128, DLOC], f32r, tag="ek")
                        nc.scalar.activation(ek, kps, Exp)
                        sk = smpool.tile([128, HLOC], f32, tag="sk")
                        nc.vector.reduce_sum(
                            sk, ek.rearrange("p (h e) -> p h e", e=HD), axis=X)
                        rk = smpool.tile([128, HLOC], f32, tag="rk")
                        nc.vector.reciprocal(rk, sk)

                        vps = pkvp.tile([128, DLOC], f32, tag="pkv")
                        for tf in range(8):
                            nc.tensor.matmul(
                                vps, xc[:, tf, t * 128:(t + 1) * 128],
                                wv_sb[:, tf, :],
                                start=(tf == 0), stop=False)
                        nc.tensor.matmul(vps, o1, bv_sb,
                                         start=False, stop=True)
                        vn = vnpool.tile([128, DLOC], f32r, tag="vn")
                        for h in range(HLOC):
                            nc.vector.tensor_scalar_mul(
                                vn[:, h * HD:(h + 1) * HD],
                                vps[:, h * HD:(h + 1) * HD], rk[:, h:h + 1])

                        first, last = (st == 0), (st == NST - 1)
                        for dst, lo, hi in ((kvA, 0, 256), (kvB, 256, 512)):
                            # start=True clears the whole 2KB psum row of
                            # every partition it writes, so only the first
                            # matmul into each bank may carry it.
                            nc.tensor.matmul(dst[:, 0:256],
                                             ek[:, lo:lo + 128],
                                             vn[:, lo:hi],
                                             start=first, stop=False,
                                             skip_group_check=True)
                            nc.tensor.matmul(dst[:, 256:512],
                                             ek[:, lo + 128:lo + 256],
                                             vn[:, lo:hi],
                                             start=False, stop=last,
                                             skip_group_check=True)

            # ---------------- phase 2 ----------------
            with ExitStack() as p2:
                wopool = p2.enter_context(tc.tile_pool(name="wo", bufs=1))
                otpool = p2.enter_context(tc.tile_pool(name="ot", bufs=2))
                rqpool = p2.enter_context(tc.tile_pool(name="rq", bufs=4))
                ysbpool = p2.enter_context(tc.tile_pool(name="ysb", bufs=3))
                opsp = p2.enter_context(
                    tc.tile_pool(name="ops", bufs=2, space="PSUM"))
                ypsp = p2.enter_context(
                    tc.tile_pool(name="yps", bufs=4, space="PSUM"))

                wo_sb = wopool.tile([128, NPAIR, DM], f32r)
                nc.sync.dma_start(
                    out=wo_sb, in_=wo.rearrange("(t p) j -> p t j", p=128))
                # zero the cross-head blocks so each 128x128 pair block of
                # KV becomes exactly block-diagonal, usable whole as lhsT
                for kvp in (kvA, kvB):
                    nc.vector.memset(kvp[0:64, 64:128], 0.0)
                    nc.vector.memset(kvp[64:128, 0:64], 0.0)
                    nc.vector.memset(kvp[0:64, 448:512], 0.0)
                    nc.vector.memset(kvp[64:128, 384:448], 0.0)
                nc.scalar.copy(kvsb[:, 0, :], kvA)
                nc.scalar.copy(kvsb[:, 1, :], kvB)
                if dbg:
                    nc.sync.dma_start(out=d_expqt, in_=expQT.bitcast(f32))
                    nc.sync.dma_start(out=d_recq, in_=recq.bitcast(f32))
                    nc.sync.dma_start(out=d_kv, in_=kvsb.bitcast(f32))

                for c in range(NCH):
                    otc = otpool.tile([128, NPAIR, SC], f32r, tag="otc")
                    for p_ in range(NPAIR):
                        ops = opsp.tile([128, SC], f32, tag="ops")
                        blk = kvsb[:, p_ // 2, 384 * (p_ % 2):
                                   384 * (p_ % 2) + 128]
                        nc.tensor.matmul(ops, blk,
                                         expQT[:, p_, c * SC:(c + 1) * SC],
                                         start=True, stop=True)
                        rqb = rqpool.tile([128, SC], f32r, tag="rqb")
                        for sub in range(2):
                            h = 2 * p_ + sub
                            src = recq[h:h + 1, c * SC:(c + 1) * SC]
                            bc = bass.AP(
                                tensor=src.tensor, offset=src.offset,
                                ap=[list(src.ap[0]), [0, 64]]
                                + [list(d) for d in src.ap[1:]])
                            nc.sync.dma_start(
                                out=rqb[64 * sub:64 * (sub + 1), :], in_=bc)
                        nc.vector.tensor_mul(otc[:, p_, :], ops, rqb)
                    if dbg:
                        nc.sync.dma_start(out=d_ot[c], in_=otc.bitcast(f32))
                    for t in range(4):
                        ysb = ysbpool.tile([128, 2, 512], f32, tag="ysb")
                        for jh in range(2):
                            yps = ypsp.tile([128, 512], f32, tag="yps")
                            for ct in range(NPAIR):
                                nc.tensor.matmul(
                                    yps,
                                    otc[:, ct, t * 128:(t + 1) * 128],
                                    wo_sb[:, ct, jh * 512:(jh + 1) * 512],
                                    start=(ct == 0), stop=(ct == NPAIR - 1))
                            nc.scalar.copy(ysb[:, jh, :], yps)
                        row = (c * 4 + t) * 128
                        nc.sync.dma_start(
                            out=y[row:row + 128, :].rearrange(
                                "p (a b) -> p a b", a=2),
                            in_=ysb)

    with tile.TileContext(nc) as tc:
        if repeat == 1:
            body(tc)
        else:
            for _ in range(repeat):
                body(tc)
    nc.compile()
    return nc


def shard_inputs(x, Wq, bq, Wk, bk, Wv, bv, Wo, S_=S):
    ones1, ones8 = make_consts()
    f = np.float32
    in_maps = []
    for core in range(NCORES):
        b, g = core // GROUPS, core % GROUPS
        sl = slice(g * DLOC, (g + 1) * DLOC)
        in_maps.append({
            "xT": np.ascontiguousarray(np.asarray(x)[b, :S_, :].T, dtype=f),
            "wq": np.ascontiguousarray(np.asarray(Wq)[:, sl], dtype=f),
            "wk": np.ascontiguousarray(np.asarray(Wk)[:, sl], dtype=f),
            "wv": np.ascontiguousarray(np.asarray(Wv)[:, sl], dtype=f),
            "wo": np.ascontiguousarray(np.asarray(Wo)[sl, :], dtype=f),
            "bq": np.asarray(bq)[sl].astype(f),
            "bk": np.asarray(bk)[sl].astype(f)[None, :],
            "bv": np.asarray(bv)[sl].astype(f)[None, :],
            "ones1": ones1,
            "ones8": ones8,
        })
    return in_maps


_NC_CACHE = {}


def _get_nc():
    if "nc" not in _NC_CACHE:
        _NC_CACHE["nc"] = build_bass()
    return _NC_CACHE["nc"]


def kernel(x, Wq, bq, Wk, bk, Wv, bv, Wo, bo):
    from concourse.bass_utils import run_bass_kernel_spmd
    nc = _get_nc()
    in_maps = shard_inputs(x, Wq, bq, Wk, bk, Wv, bv, Wo)
    res = run_bass_kernel_spmd(nc, in_maps, list(range(NCORES)))
    parts = [res.results[i]["y"] for i in range(NCORES)]
    out = np.stack([parts[2 * b] + parts[2 * b + 1] for b in range(B)])
    out += np.asarray(bo, dtype=np.float32)
    return out.astype(np.float32)


def oracle_core(inp, S_=S):
    """Numpy mirror of the per-core computation, for debugging."""
    xT = inp["xT"].astype(np.float64)
    Q = xT.T @ inp["wq"] + inp["bq"]
    K = xT.T @ inp["wk"] + inp["bk"][0]
    V = xT.T @ inp["wv"] + inp["bv"][0]
    out = np.zeros((S_, DLOC))
    for h in range(HLOC):
        sl = slice(h * HD, (h + 1) * HD)
        eq, ek = np.exp(Q[:, sl]), np.exp(K[:, sl])
        qh = eq / eq.sum(-1, keepdims=True)
        kh = ek / ek.sum(-1, keepdims=True)
        out[:, sl] = qh @ (kh.T @ V[:, sl])
    return (out @ inp["wo"]).astype(np.float32)


# revision 15
# speedup vs baseline: 214.4405x; 214.4405x over previous
"""Linear-attention Trainium2 kernel (8 NeuronCores, SPMD).

Sharding: batch (4) x head-group (2). Core i handles batch i//2, heads
[8*(i%2), 8*(i%2)+8). Each core computes its partial output through Wo;
the host sums the two partials per batch and adds bo.

Per-core dataflow (all matmuls in float32r):
  xT = x[b].T                                   [1024, 4096]   (host transpose)
  Q^T = Wq_g^T-contract xT  (PE, f on parts)    [512, 4096]    d on partitions
  expQ^T = exp(Q^T + bq)    (ACT, bias/part)
  sQ    = blockdiag-ones matmul colsums         [8, 4096]
  K     = xT^T-contract Wk_g (natural layout)   [4096, 512]    s on partitions
  expK  = exp(K + bk)       (ACT; bias via k=1 outer-product matmul)
  V'    = (V + bv) * 1/rowsum(expK) per head    (DVE tensor_scalar per head)
  KV_h  = expK_h^T @ V'_h   (PE, accumulated in PSUM over all of S)
  out^T_h = KV_h^T-contract expQ^T_h, then * (1/sQ) via DMA-broadcast + DVE
  y_partial = out^T^T-contract Wo_g             [4096, 1024]
"""

import numpy as np

B, S, DM, H = 4, 4096, 1024, 16
HD = 64
GROUPS = 2
DLOC = DM // GROUPS   # 512 channels per core
HLOC = H // GROUPS    # 8 heads per core
NCORES = B * GROUPS   # 8
SC = 512              # sequence chunk


def make_consts():
    ones1 = np.ones((1, 128), np.float32)
    ones8 = np.zeros((128, 4 * HLOC), np.float32)
    for dt_ in range(4):  # pair-tile index
        for sub in range(2):
            ones8[64 * sub:64 * (sub + 1), dt_ * HLOC + 2 * dt_ + sub] = 1.0
    return ones1, ones8


def kv_region(h):
    """(half, row_base, col_base) of KV_h inside kv psum tile [128, 2, 512]."""
    return h // 4, 64 * (h % 2), 256 * ((h // 2) % 2) + 64 * (h % 4)


def build_bass(S_=S, n_devices=NCORES, repeat=1, dbg=False):
    from contextlib import ExitStack
    import concourse.bass as bass
    import concourse.bacc as bacc
    import concourse.mybir as mybir
    import concourse.tile as tile

    f32 = mybir.dt.float32
    f32r = mybir.dt.float32r
    Exp = mybir.ActivationFunctionType.Exp
    X = mybir.AxisListType.X

    NCH = S_ // SC        # sequence chunks
    NPAIR = DLOC // 128   # 4 pair-tiles (2 heads each)
    NST = S_ // 128       # sequence tiles

    nc = bacc.Bacc("TRN2", target_bir_lowering=False, debug=False,
                   num_devices=n_devices)
    xT = nc.dram_tensor("xT", [DM, S_], f32r, kind="ExternalInput").ap()
    wq = nc.dram_tensor("wq", [DM, DLOC], f32r, kind="ExternalInput").ap()
    wk = nc.dram_tensor("wk", [DM, DLOC], f32r, kind="ExternalInput").ap()
    wv = nc.dram_tensor("wv", [DM, DLOC], f32r, kind="ExternalInput").ap()
    wo = nc.dram_tensor("wo", [DLOC, DM], f32r, kind="ExternalInput").ap()
    bq = nc.dram_tensor("bq", [DLOC], f32, kind="ExternalInput").ap()
    bk = nc.dram_tensor("bk", [1, DLOC], f32r, kind="ExternalInput").ap()
    bv = nc.dram_tensor("bv", [1, DLOC], f32r, kind="ExternalInput").ap()
    ones1 = nc.dram_tensor("ones1", [1, 128], f32r, kind="ExternalInput").ap()
    ones8 = nc.dram_tensor("ones8", [128, 4 * HLOC], f32r,
                           kind="ExternalInput").ap()
    y = nc.dram_tensor("y", [S_, DM], f32, kind="ExternalOutput").ap()
    NPAIR_ = DLOC // 128
    if dbg:
        d_expqt = nc.dram_tensor("d_expqt", [128, NPAIR_, S_], f32,
                                 kind="ExternalOutput").ap()
        d_recq = nc.dram_tensor("d_recq", [HLOC, S_], f32,
                                kind="ExternalOutput").ap()
        d_kv = nc.dram_tensor("d_kv", [128, 2, 512], f32,
                              kind="ExternalOutput").ap()
        d_ot = nc.dram_tensor("d_ot", [S_ // SC, 128, NPAIR_, SC], f32,
                              kind="ExternalOutput").ap()

    xTr = xT.rearrange("(tf p) s -> p tf s", p=128)

    def body(tc):
        ctx = ExitStack()
        with ctx:
            cons = ctx.enter_context(tc.tile_pool(name="cons", bufs=1))
            persist = ctx.enter_context(tc.tile_pool(name="persist", bufs=1))
            kvpsp = ctx.enter_context(
                tc.tile_pool(name="kvps", bufs=1, space="PSUM"))

            bqT = cons.tile([128, NPAIR], f32)
            nc.sync.dma_start(out=bqT, in_=bq.rearrange("(t p) -> p t", p=128))
            bk_sb = cons.tile([1, DLOC], f32r)
            nc.sync.dma_start(out=bk_sb, in_=bk)
            bv_sb = cons.tile([1, DLOC], f32r)
            nc.sync.dma_start(out=bv_sb, in_=bv)
            o1 = cons.tile([1, 128], f32r)
            nc.sync.dma_start(out=o1, in_=ones1)
            o8 = cons.tile([128, 4 * HLOC], f32r)
            nc.sync.dma_start(out=o8, in_=ones8)

            expQT = persist.tile([128, NPAIR, S_], f32r)
            recq = persist.tile([HLOC, S_], f32r)
            kvsb = persist.tile([128, 2, 512], f32r)
            kvA = kvpsp.tile([128, 512], f32, tag="kvA")
            kvB = kvpsp.tile([128, 512], f32, tag="kvB")

            # ---------------- phase 1 ----------------
            with ExitStack() as p1:
                wpool = p1.enter_context(tc.tile_pool(name="wqkv", bufs=1))
                xpool = p1.enter_context(tc.tile_pool(name="xc", bufs=2))
                ekpool = p1.enter_context(tc.tile_pool(name="ek", bufs=4))
                vnpool = p1.enter_context(tc.tile_pool(name="vn", bufs=4))
                smpool = p1.enter_context(tc.tile_pool(name="sm", bufs=4))
                qpsp = p1.enter_context(
                    tc.tile_pool(name="qps", bufs=2, space="PSUM"))
                sqpsp = p1.enter_context(
                    tc.tile_pool(name="sqps", bufs=1, space="PSUM"))
                pkvp = p1.enter_context(
                    tc.tile_pool(name="pkv", bufs=3, space="PSUM"))

                wq_sb = wpool.tile([128, 8, DLOC], f32r, tag="wq")
                nc.sync.dma_start(
                    out=wq_sb, in_=wq.rearrange("(tf p) d -> p tf d", p=128))
                wk_sb = wpool.tile([128, 8, DLOC], f32r, tag="wk")
                nc.sync.dma_start(
                    out=wk_sb, in_=wk.rearrange("(tf p) d -> p tf d", p=128))
                wv_sb = wpool.tile([128, 8, DLOC], f32r, tag="wv")
                nc.sync.dma_start(
                    out=wv_sb, in_=wv.rearrange("(tf p) d -> p tf d", p=128))

                for c in range(NCH):
                    xc = xpool.tile([128, 8, SC], f32r, tag="xc")
                    nc.sync.dma_start(out=xc,
                                      in_=xTr[:, :, c * SC:(c + 1) * SC])
                    # Q^T pair-tiles + exp + column sums
                    sqps = sqpsp.tile([HLOC, SC], f32, tag="sq")
                    for dt_ in range(NPAIR):
                        qps = qpsp.tile([128, SC], f32, tag="q")
                        for tf in range(8):
                            nc.tensor.matmul(
                                qps, wq_sb[:, tf, dt_ * 128:(dt_ + 1) * 128],
                                xc[:, tf, :],
                                start=(tf == 0), stop=(tf == 7))
                        eq = expQT[:, dt_, c * SC:(c + 1) * SC]
                        nc.scalar.activation(eq, qps, Exp,
                                             bias=bqT[:, dt_:dt_ + 1],
                                             scale=1.0)
                        nc.tensor.matmul(
                            sqps, o8[:, dt_ * HLOC:(dt_ + 1) * HLOC], eq,
                            start=(dt_ == 0), stop=(dt_ == NPAIR - 1))
                    with nc.allow_low_precision(reason="f32r rounding ok"):
                        nc.vector.reciprocal(
                            recq[:, c * SC:(c + 1) * SC], sqps)

                    # K / V / KV per 128-row sequence tile
                    for t in range(4):
                        st = c * 4 + t
                        kps = pkvp.tile([128, DLOC], f32, tag="pkv")
                        for tf in range(8):
                            nc.tensor.matmul(
                                kps, xc[:, tf, t * 128:(t + 1) * 128],
                                wk_sb[:, tf, :],
                                start=(tf == 0), stop=False)
                        nc.tensor.matmul(kps, o1, bk_sb,
                                         start=False, stop=True)
                        ek = ekpool.tile([128, DLOC], f32r, tag="ek")
                        nc.scalar.activation(ek, kps, Exp)
                        sk = smpool.tile([128, HLOC], f32, tag="sk")
                        nc.vector.reduce_sum(
                            sk, ek.rearrange("p (h e) -> p h e", e=HD), axis=X)
                        rk = smpool.tile([128, HLOC], f32, tag="rk")
                        nc.vector.reciprocal(rk, sk)

                        vps = pkvp.tile([128, DLOC], f32, tag="pkv")
                        for tf in range(8):
                            nc.tensor.matmul(
                                vps, xc[:, tf, t * 128:(t + 1) * 128],
                                wv_sb[:, tf, :],
                                start=(tf == 0), stop=False)
                        nc.tensor.matmul(vps, o1, bv_sb,
                                         start=False, stop=True)
                        vn = vnpool.tile([128, DLOC], f32r, tag="vn")
                        rkb = bass.AP(
                            tensor=rk.tensor, offset=rk.offset,
                            ap=[list(rk.ap[0]), [1, HLOC], [0, HD]])
                        nc.vector.tensor_tensor(
                            out=vn.rearrange("p (h e) -> p h e", e=HD),
                            in0=vps.rearrange("p (h e) -> p h e", e=HD),
                            in1=rkb, op=mybir.AluOpType.mult)

                        first, last = (st == 0), (st == NST - 1)
                        for dst, lo, hi in ((kvA, 0, 256), (kvB, 256, 512)):
                            # start=True clears the whole 2KB psum row of
                            # every partition it writes, so only the first
                            # matmul into each bank may carry it.
                            nc.tensor.matmul(dst[:, 0:256],
                                             ek[:, lo:lo + 128],
                                             vn[:, lo:hi],
                                             start=first, stop=False,
                                             skip_group_check=True)
                            nc.tensor.matmul(dst[:, 256:512],
                                             ek[:, lo + 128:lo + 256],
                                             vn[:, lo:hi],
                                             start=False, stop=last,
                                             skip_group_check=True)

            # ---------------- phase 2 ----------------
            with ExitStack() as p2:
                wopool = p2.enter_context(tc.tile_pool(name="wo", bufs=1))
                otpool = p2.enter_context(tc.tile_pool(name="ot", bufs=2))
                rqpool = p2.enter_context(tc.tile_pool(name="rq", bufs=8))
                ysbpool = p2.enter_context(tc.tile_pool(name="ysb", bufs=3))
                opsp = p2.enter_context(
                    tc.tile_pool(name="ops", bufs=2, space="PSUM"))
                ypsp = p2.enter_context(
                    tc.tile_pool(name="yps", bufs=4, space="PSUM"))

                wo_sb = wopool.tile([128, NPAIR, DM], f32r)
                nc.sync.dma_start(
                    out=wo_sb, in_=wo.rearrange("(t p) j -> p t j", p=128))
                # zero the cross-head blocks so each 128x128 pair block of
                # KV becomes exactly block-diagonal, usable whole as lhsT
                for kvp in (kvA, kvB):
                    nc.vector.memset(kvp[0:64, 64:128], 0.0)
                    nc.vector.memset(kvp[64:128, 0:64], 0.0)
                    nc.vector.memset(kvp[0:64, 448:512], 0.0)
                    nc.vector.memset(kvp[64:128, 384:448], 0.0)
                nc.scalar.copy(kvsb[:, 0, :], kvA)
                nc.scalar.copy(kvsb[:, 1, :], kvB)
                if dbg:
                    nc.sync.dma_start(out=d_expqt, in_=expQT.bitcast(f32))
                    nc.sync.dma_start(out=d_recq, in_=recq.bitcast(f32))
                    nc.sync.dma_start(out=d_kv, in_=kvsb.bitcast(f32))

                for c in range(NCH):
                    otc = otpool.tile([128, NPAIR, SC], f32r, tag="otc")
                    for p_ in range(NPAIR):
                        ops = opsp.tile([128, SC], f32, tag="ops")
                        blk = kvsb[:, p_ // 2, 384 * (p_ % 2):
                                   384 * (p_ % 2) + 128]
                        nc.tensor.matmul(ops, blk,
                                         expQT[:, p_, c * SC:(c + 1) * SC],
                                         start=True, stop=True)
                        rqb = rqpool.tile([128, SC], f32r, tag="rqb")
                        for sub in range(2):
                            h = 2 * p_ + sub
                            src_ = recq[h:h + 1, c * SC:(c + 1) * SC]
                            bc = bass.AP(
                                tensor=src_.tensor, offset=src_.offset,
                                ap=[list(src_.ap[0]), [0, 64]]
                                + [list(d) for d in src_.ap[1:]])
                            nc.sync.dma_start(
                                out=rqb[64 * sub:64 * (sub + 1), :], in_=bc)
                        nc.vector.tensor_mul(otc[:, p_, :], ops, rqb)
                    if dbg:
                        nc.sync.dma_start(out=d_ot[c], in_=otc.bitcast(f32))
                    for t in range(4):
                        ysb = ysbpool.tile([128, 2, 512], f32, tag="ysb")
                        for jh in range(2):
                            yps = ypsp.tile([128, 512], f32, tag="yps")
                            for ct in range(NPAIR):
                                nc.tensor.matmul(
                                    yps,
                                    otc[:, ct, t * 128:(t + 1) * 128],
                                    wo_sb[:, ct, jh * 512:(jh + 1) * 512],
                                    start=(ct == 0), stop=(ct == NPAIR - 1))
                            nc.scalar.copy(ysb[:, jh, :], yps)
                        row = (c * 4 + t) * 128
                        nc.sync.dma_start(
                            out=y[row:row + 128, :].rearrange(
                                "p (a b) -> p a b", a=2),
                            in_=ysb)

    with tile.TileContext(nc) as tc:
        if repeat == 1:
            body(tc)
        else:
            for _ in range(repeat):
                body(tc)
    nc.compile()
    return nc


def shard_inputs(x, Wq, bq, Wk, bk, Wv, bv, Wo, S_=S):
    ones1, ones8 = make_consts()
    f = np.float32
    in_maps = []
    for core in range(NCORES):
        b, g = core // GROUPS, core % GROUPS
        sl = slice(g * DLOC, (g + 1) * DLOC)
        in_maps.append({
            "xT": np.ascontiguousarray(np.asarray(x)[b, :S_, :].T, dtype=f),
            "wq": np.ascontiguousarray(np.asarray(Wq)[:, sl], dtype=f),
            "wk": np.ascontiguousarray(np.asarray(Wk)[:, sl], dtype=f),
            "wv": np.ascontiguousarray(np.asarray(Wv)[:, sl], dtype=f),
            "wo": np.ascontiguousarray(np.asarray(Wo)[sl, :], dtype=f),
            "bq": np.asarray(bq)[sl].astype(f),
            "bk": np.asarray(bk)[sl].astype(f)[None, :],
            "bv": np.asarray(bv)[sl].astype(f)[None, :],
            "ones1": ones1,
            "ones8": ones8,
        })
    return in_maps


_NC_CACHE = {}


def _get_nc():
    if "nc" not in _NC_CACHE:
        _NC_CACHE["nc"] = build_bass()
    return _NC_CACHE["nc"]


def kernel(x, Wq, bq, Wk, bk, Wv, bv, Wo, bo):
    from concourse.bass_utils import run_bass_kernel_spmd
    nc = _get_nc()
    in_maps = shard_inputs(x, Wq, bq, Wk, bk, Wv, bv, Wo)
    res = run_bass_kernel_spmd(nc, in_maps, list(range(NCORES)))
    parts = [res.results[i]["y"] for i in range(NCORES)]
    out = np.stack([parts[2 * b] + parts[2 * b + 1] for b in range(B)])
    out += np.asarray(bo, dtype=np.float32)
    return out.astype(np.float32)


def oracle_core(inp, S_=S):
    """Numpy mirror of the per-core computation, for debugging."""
    xT = inp["xT"].astype(np.float64)
    Q = xT.T @ inp["wq"] + inp["bq"]
    K = xT.T @ inp["wk"] + inp["bk"][0]
    V = xT.T @ inp["wv"] + inp["bv"][0]
    out = np.zeros((S_, DLOC))
    for h in range(HLOC):
        sl = slice(h * HD, (h + 1) * HD)
        eq, ek = np.exp(Q[:, sl]), np.exp(K[:, sl])
        qh = eq / eq.sum(-1, keepdims=True)
        kh = ek / ek.sum(-1, keepdims=True)
        out[:, sl] = qh @ (kh.T @ V[:, sl])
    return (out @ inp["wo"]).astype(np.float32)
